# revision 32
# baseline (speedup 1.0000x reference)
"""Trainium2 Bass kernel for nn_Attention (buggy-reshape attention), 8-core SPMD.

Math (reference): q/k/v = (x @ W).reshape entangles batch and head. Each of the
256 (h,b) "chunks" is a contiguous 64-row block of the projected (16384, 512)
matrices:
  K_c = XK[64c:64c+64, :]            (64=A, 512=M)  -- used as-is
  Q_c = XQ[64c:64c+64, :].reshape(512, 64)
  V_c = XV[64c:64c+64, :].reshape(512, 64)
  out_c = softmax(Q_c @ K_c, -1) @ V_c ; final[b] = relu(mean_h out_(h,b) + x_b @ Wr)

Chunk (h,b) touches only x[4h + b//8, 64*(b%8):64*(b%8)+64, :]. We shard by
OUTPUT batch: core d owns batches 4d..4d+3 (all 8 heads) and is handed exactly
the x rows it needs -> zero collectives; head-mean is local.

Per-core layouts (m-permutation p = 64*s + r where m = 8*r + s; same perm used
for the n axis via host-permuted Wk columns):
  S^T tiles (n'-part, p-free) = Ksb_slice.T @ QTall_slice ; softmax over n'
  (partition axis; no max subtraction -- scores are O(+-50), exp fits fp32
  easily); column sums via ones-matmul; O^T = V_perm.T @ expS, normalized by
  approx-reciprocal broadcast; 1/8 head-mean folded into Wv.

Schedule: the Act engine's exp stream is the metronome (4 x [128,1024] exps
per chunk-pair, ~4.2us -- Act has no 2x mode).  Per iteration the 8 S-matmuls
are emitted as four 2-matmul quanta, each immediately followed by its exp and
separated by ~0.6us filler blocks of other PE work (K/V/Q projections, column
sums, O matmuls), so PE runs continuously (keeping its p-state at 2.4GHz) and
Act never starves.  exp pre-sums run on the otherwise-idle gpsimd engine
((kn0+kn1), (kn2+kn3), each depending on a single exp); DVE folds them and the
column-sum is a single accumulation chain per chunk.  All psum->sbuf drains
live on Act/DVE (gpsimd cannot touch psum).

dtypes: everything fp16 into the PE except the exp output (bf16, needs range)
and so the V/ones stationaries of the O/sum matmuls are bf16 to match es;
psum always fp32; final output fp16 (cast to fp32 on host).  M=64 matmuls are
column-paired onto disjoint PE halves via psum base-partition-64 outputs (auto
tile_position); two accumulation chains never interleave within the same psum
(bank, partition-half).  Row-pairing the M=512 S-matmuls is deliberately
avoided (trips the chip power governor, ~20% global clock drop, measured).
"""

import os
import sys

import numpy as np

sys.path.insert(0, "/opt/trn_rl_repo")

import concourse.bass as bass
import concourse.bacc as bacc
import concourse.mybir as mybir
from concourse.tile import TileContext

FP = mybir.dt.float32
BF = mybir.dt.bfloat16
F16 = mybir.dt.float16
AF = mybir.ActivationFunctionType
ALU = mybir.AluOpType

B, M, E, H, A = 32, 512, 256, 8, 64
NCORES = 8

# m (and n) permutation: p = 64*s + r  <->  m = 8*r + s
_M_OF_P = np.array([8 * (p % 64) + p // 64 for p in range(512)])
_P_OF_M = np.array([64 * (m % 8) + m // 8 for m in range(512)])


def build_core_graph():
    nc = bacc.Bacc(target_bir_lowering=False)

    xaT_e = nc.declare_dram_parameter("xaT", [E, 2048], F16, isOutput=False)
    xoT_e = nc.declare_dram_parameter("xoT", [E, 2048], F16, isOutput=False)
    wqk_e = nc.declare_dram_parameter("wqk", [E, 1024], F16, isOutput=False)
    wvr_e = nc.declare_dram_parameter("wvr", [E, 512 + A], F16, isOutput=False)
    out_e = nc.declare_dram_parameter("out", [A, 2048], F16, isOutput=True)

    with TileContext(nc) as tc:
        from contextlib import ExitStack

        with ExitStack() as ctx:
            const = ctx.enter_context(tc.tile_pool(name="const", bufs=1))
            qt_pool = ctx.enter_context(tc.tile_pool(name="qt", bufs=2))
            ksb_pool = ctx.enter_context(tc.tile_pool(name="ksb", bufs=8))
            vsb_pool = ctx.enter_context(tc.tile_pool(name="vsb", bufs=8))
            exps_pool = ctx.enter_context(tc.tile_pool(name="exps", bufs=4))
            esum_pool = ctx.enter_context(tc.tile_pool(name="esum", bufs=4))
            esum1_pool = ctx.enter_context(tc.tile_pool(name="esum1", bufs=2))
            misc_pool = ctx.enter_context(tc.tile_pool(name="misc", bufs=4))
            acc_pool = ctx.enter_context(tc.tile_pool(name="acc", bufs=2))

            pp_psum = ctx.enter_context(tc.tile_pool(name="pp", bufs=2, space="PSUM"))
            st_psum = ctx.enter_context(tc.tile_pool(name="st", bufs=2, space="PSUM"))
            so_psum = ctx.enter_context(tc.tile_pool(name="so", bufs=2, space="PSUM"))

            # ---- input tiles ----
            xaT_t = const.tile([128, 2, 4, 4, 2, A], F16, tag="xaT")
            xoT_t = const.tile([128, 2, 2048], F16, tag="xoT")
            wqk = const.tile([128, 2, 1024], F16, tag="wqk")
            wvr = const.tile([128, 2, 512 + A], F16, tag="wvr")
            ones = const.tile([128, A], BF, tag="ones")
            nc.vector.memset(ones[:], 1.0)

            # startup-critical loads, finest first: the first K matmul needs
            # only wkp + xaT(g0,c0); the par0 Q projection then needs wq +
            # the rest of xaT g0.  wvr (V/Wr weights) follows; bulk (xaT
            # g1:3, xoT) is gated behind group-0 compute below so it cannot
            # steal DMA engines from these.
            nc.sync.dma_start(
                out=xaT_t[:, :, 0, 0, :, :],
                in_=xaT_e[:, 0:128].rearrange("(k p) (t r) -> p k t r",
                                              k=2, t=2))
            nc.scalar.dma_start(
                out=wqk[:, :, 512:1024],
                in_=wqk_e[:, 512:1024].rearrange("(k p) n -> p k n", k=2))
            nc.sync.dma_start(
                out=wqk[:, :, 0:512],
                in_=wqk_e[:, 0:512].rearrange("(k p) n -> p k n", k=2))
            nc.sync.dma_start(
                out=xaT_t[:, :, 0, 1:4, :, :],
                in_=xaT_e[:, 128:512].rearrange("(k p) (c t r) -> p k c t r",
                                                k=2, c=3, t=2))
            nc.scalar.dma_start(
                out=wvr[:, :, :],
                in_=wvr_e[:, :].rearrange("(k p) n -> p k n", k=2))

            wq = [wqk[:, k, 0:512] for k in range(2)]
            wkp = [wqk[:, k, 512:1024] for k in range(2)]
            wv8 = [wvr[:, k, 0:512] for k in range(2)]
            wv8v = [wv8[k].rearrange("p (hh sp a) -> p hh sp a", hh=4, sp=2)
                    for k in range(2)]
            wr = [wvr[:, k, 512:512 + A] for k in range(2)]
            xoT = [xoT_t[:, k, :] for k in range(2)]

            acc2 = [acc_pool.tile([128, 512], FP, tag="acc", name="acc")
                    for _ in range(2)]
            prt2 = const.tile([128, 2, 512], FP, tag="prt2")

            qtall_t = {}
            ksb_t = {}
            vsb_t = {}
            es_t = {}
            esum_t = {}
            esum1_t = {}
            sum_ps = {}

            # ---------------- building blocks ----------------
            def q_proj_par(g, sp, par, drain):
                # one psum bank holds two s-blocks (sh) of one par half; the
                # two accumulation chains are strictly sequential and one
                # copy drains both -> half the psum handoffs.
                if sp == 0 and par == 0:
                    qtall_t[g] = qt_pool.tile([128, 8, 4, A], F16, tag="qt",
                                              name="qtall")
                qp4 = pp_psum.tile([128, 2, 4, A], FP, tag="pp", name="qp4")
                for sh in range(2):
                    s = 2 * sp + sh
                    for k in range(2):
                        nc.tensor.matmul(
                            qp4[64 * par:64 * par + 64, sh, :, :],
                            wq[k][:, 64 * s:64 * s + 64],
                            xaT_t[:, k, g, :, par, :],
                            start=(k == 0), stop=(k == 1),
                            skip_group_check=True)
                dst = qtall_t[g][64 * par:64 * par + 64, 2 * sp:2 * sp + 2]
                src = qp4[64 * par:64 * par + 64, :, :, :]
                if drain == "act":
                    nc.scalar.copy(dst, src)
                else:
                    nc.vector.tensor_copy(dst, src)

            def q_proj2(g, sp, drain="dve", pool=None):
                # both par halves in one psum tile, column-paired matmuls
                if sp == 0:
                    qtall_t[g] = qt_pool.tile([128, 8, 4, A], F16, tag="qt",
                                              name="qtall")
                pool = pool or pp_psum
                qp4 = pool.tile([128, 2, 4, A], FP,
                                tag="pp" if pool is pp_psum else "so",
                                name="qp4")
                for sh in range(2):
                    s = 2 * sp + sh
                    for k in range(2):
                        for par in range(2):
                            nc.tensor.matmul(
                                qp4[64 * par:64 * par + 64, sh, :, :],
                                wq[k][:, 64 * s:64 * s + 64],
                                xaT_t[:, k, g, :, par, :],
                                start=(k == 0), stop=(k == 1),
                                skip_group_check=True)
                if drain == "act":
                    nc.scalar.copy(qtall_t[g][:, 2 * sp:2 * sp + 2, :, :],
                                   qp4[:, :, :, :])
                else:
                    nc.vector.tensor_copy(
                        qtall_t[g][:, 2 * sp:2 * sp + 2, :, :],
                        qp4[:, :, :, :])

            def k_proj(g, c):
                kp2 = pp_psum.tile([128, 512], FP, tag="pp", name="kp2")
                for k in range(2):
                    for par in range(2):
                        nc.tensor.matmul(kp2[64 * par:64 * par + 64, :],
                                         xaT_t[:, k, g, c, par, :], wkp[k],
                                         start=(k == 0), stop=(k == 1),
                                         skip_group_check=True)
                ksb_t[(g, c)] = ksb_pool.tile([128, 512], F16, tag="ksb",
                                              name="ksb")
                nc.vector.tensor_copy(ksb_t[(g, c)][:], kp2[:])

            pv_ps = {}

            def v_mm(g, c, phase):
                # psum lands directly in V_perm layout (sp -> partition half,
                # par -> free dim); chains (0,0)/(1,1) complete before
                # (0,1)/(1,0) so no psum partition-half hosts two chains.
                # Split in two 4mm phases so the block packs into slim slots.
                if phase == 0:
                    pv_ps[(g, c)] = pp_psum.tile([128, 4, 2, A], FP,
                                                 tag="pp", name="pv4")
                pv4 = pv_ps[(g, c)]
                grp = (((0, 0), (1, 1)), ((0, 1), (1, 0)))[phase]
                for k in range(2):
                    for par, sp in grp:
                        nc.tensor.matmul(
                            pv4[64 * sp:64 * sp + 64, :, par, :],
                            xaT_t[:, k, g, c, par, :],
                            wv8v[k][:, :, sp, :],
                            start=(k == 0), stop=(k == 1),
                            skip_group_check=True)
                if phase == 1:
                    pv_ps.pop((g, c))
                    vsb = vsb_pool.tile([128, 4, 2, A], BF, tag="vsb",
                                        name="vsb")
                    nc.scalar.copy(vsb[:, 0:2], pv4[:, 0:2])
                    nc.vector.tensor_copy(vsb[:, 2:4], pv4[:, 2:4])
                    vsb_t[(g, c)] = vsb

            def s_half(g, c, par, half):
                # 2 S^T matmuls + 1 exp for one (par, half).  Unpaired on the
                # PE row halves on purpose: pairing trips the chip power
                # governor (~20% global clock drop, measured).
                if par == 0 and half == 0:
                    es_t[(g, c)] = {}
                if half == 0:
                    es = exps_pool.tile([128, 4, 512], BF, tag="exps",
                                        name="es")
                    es_t[(g, c)][par] = es
                else:
                    es = es_t[(g, c)][par]
                ksb = ksb_t[(g, c)]
                qtall = qtall_t[g]
                st = st_psum.tile([128, 2, 512], FP, tag="st", name="st")
                for q2 in range(2):
                    kn = 2 * half + q2
                    nc.tensor.matmul(
                        st[:, q2, :],
                        ksb[64 * par:64 * par + 64,
                            128 * kn:128 * kn + 128],
                        qtall[64 * par:64 * par + 64, :, c, :],
                        start=True, stop=True)
                nc.scalar.activation(es[:, 2 * half:2 * half + 2, :],
                                     st[:], AF.Exp)

            def presum1(g, c, par, half, engine):
                # esum[:, half, :] = es[:, 2h, :] + es[:, 2h+1, :] -- depends
                # on exactly one exp; runs on gpsimd (idle) in steady state.
                if half == 0:
                    esum_t.setdefault((g, c), {})[par] = esum_pool.tile(
                        [128, 2, 512], BF, tag="esum", name="esum")
                es = es_t[(g, c)][par]
                esum = esum_t[(g, c)][par]
                eng = nc.gpsimd if engine == "pool" else nc.vector
                eng.tensor_add(esum[:, half, :], es[:, 2 * half, :],
                               es[:, 2 * half + 1, :])

            def presum2(g, c):
                # fold to a single kn chunk on DVE -> single-chain column sum
                esum1_t[(g, c)] = {}
                for par in range(2):
                    esum = esum_t[(g, c)].pop(par)
                    e1 = esum1_pool.tile([128, 512], BF, tag="esum1",
                                         name="esum1")
                    esum1_t[(g, c)][par] = e1
                    nc.vector.tensor_add(e1[:], esum[:, 0, :], esum[:, 1, :])

            def colsum(g, c):
                sumb2 = so_psum.tile([128, 512], FP, tag="so", name="sumb2")
                sum_ps[(g, c)] = sumb2
                e1 = esum1_t.pop((g, c))
                for par in range(2):
                    nc.tensor.matmul(sumb2[64 * par:64 * par + 64, :],
                                     ones[:, 0:A], e1[par][:],
                                     start=True, stop=True,
                                     skip_group_check=True)

            rec_t = {}

            def recip(g, c):
                sumb2 = sum_ps[(g, c)]
                recipb2 = misc_pool.tile([128, 512], FP, tag="recip",
                                         name="recipb2")
                nc.vector.reciprocal_approx_fast(out=recipb2[:],
                                                 in_=sumb2[:])
                rec_t[(g, c)] = recipb2

            ot_ps = {}

            def o_mm_p(g, c, par):
                # one par's 4mm accumulation chain; par 0 runs at the end of
                # one iteration, par 1 at the start of the next, so the O
                # block never fattens a single inter-quantum slot.
                if par == 0:
                    ot_ps[(g, c)] = so_psum.tile([128, 512], FP, tag="so",
                                                 name="ot2")
                ot2 = ot_ps[(g, c)]
                es = es_t[(g, c)][par]
                vsb = vsb_t[(g, c)]
                for kn in range(4):
                    nc.tensor.matmul(ot2[64 * par:64 * par + 64, :],
                                     vsb[:, kn, par, :], es[:, kn, :],
                                     start=(kn == 0), stop=(kn == 3),
                                     skip_group_check=True)
                if par == 1:
                    es_t.pop((g, c))
                    vsb_t.pop((g, c))

            def normalize(g, c):
                ot2 = ot_ps.pop((g, c))
                recipb2 = rec_t.pop((g, c))
                q = c % 2
                first = (g == 0 and c < 2)
                sum_ps.pop((g, c))
                if first:
                    nc.vector.tensor_mul(acc2[q][:], ot2[:], recipb2[:])
                else:
                    # mul reads psum (DVE only); the accumulate is sbuf-only
                    # and runs on gpsimd, which has slack
                    otmp2 = misc_pool.tile([128, 512], FP, tag="otmp",
                                           name="otmp2")
                    nc.vector.tensor_mul(otmp2[:], ot2[:], recipb2[:])
                    eng = nc.vector if (g, c) == (3, 2) else nc.gpsimd
                    eng.tensor_add(acc2[q][:], acc2[q][:], otmp2[:])

            def wr_proj(q):
                rp2 = pp_psum.tile([128, 512], FP, tag="pp", name="rp2")
                for k in range(2):
                    for par in range(2):
                        nc.tensor.matmul(
                            rp2[64 * par:64 * par + 64, :],
                            wr[k],
                            xoT[k][:, 512 * (2 * q + par):
                                   512 * (2 * q + par) + 512],
                            start=(k == 0), stop=(k == 1),
                            skip_group_check=True)
                nc.vector.tensor_copy(prt2[:, q, :], rp2[:])

            def epilogue(q):
                # acc2[q] is complete; pre-add on gpsimd (sbuf-only), relu on
                # DVE (gpsimd max with dtype cast is pathologically slow)
                outsb2 = misc_pool.tile([128, 512], F16, tag="outsb",
                                        name="outsb2")
                pre2 = misc_pool.tile([128, 512], FP, tag="pre", name="pre2")
                nc.gpsimd.tensor_add(pre2[:], acc2[q][:], prt2[:, q, :])
                nc.vector.tensor_scalar_max(outsb2[:], pre2[:], 0.0)
                for par in range(2):
                    nc.gpsimd.dma_start(
                        out=out_e[:, 512 * (2 * q + par):
                                  512 * (2 * q + par) + 512],
                        in_=outsb2[64 * par:64 * par + 64, :])

            # ---------------- prologue: pair (0,0) + group-0 projections ----
            # PE p-state warmup during the input-DMA wait: the ramp reaches
            # 2.4GHz only after ~3us of continuous execution, so burn dummy
            # matmuls (never read) while the weights stream in; the real
            # projections then start at full clock.
            warmsrc = const.tile([128, 512], BF, tag="warmsrc")
            nc.vector.memset(warmsrc[:], 1.0)
            warm = so_psum.tile([128, 512], FP, tag="so", name="warm")
            for _ in range(12):
                nc.tensor.matmul(warm[0:64, :], ones[:, 0:A], warmsrc[:],
                                 start=True, stop=True, skip_group_check=True)
            k_proj(0, 0)
            for sp in range(4):
                # alternate the psum pool so the four blocks aren't gated on
                # each other's drains through the 2-buf pp rotation
                q_proj2(0, sp, drain="act" if sp % 2 == 0 else "dve",
                        pool=pp_psum if sp % 2 == 0 else so_psum)
                if sp == 0:
                    # Bulk loads gated behind early group-0 compute via junk
                    # WAW stores (overwritten by the DMAs) so their transfers
                    # don't steal DMA engines from the startup-critical ones.
                    nc.gpsimd.tensor_copy(xaT_t[0:1, 0, 1, 0, 0, 0:4],
                                          qtall_t[0][0:1, 0, 0, 0:4])
                    nc.gpsimd.tensor_copy(xoT_t[0:1, 0, 0:4],
                                          qtall_t[0][0:1, 0, 0, 0:4])
                    nc.gpsimd.dma_start(
                        out=xaT_t[:, :, 1:4, :, :, :],
                        in_=xaT_e[:, 512:2048].rearrange(
                            "(k p) (g c t r) -> p k g c t r", k=2, g=3, c=4,
                            t=2))
                    nc.gpsimd.dma_start(
                        out=xoT_t[:, :, :],
                        in_=xoT_e[:, :].rearrange("(k p) n -> p k n", k=2))
            # first exps fire as early as possible; pair (0,0)'s V
            # projection fills PE between the first quanta
            s_half(0, 0, 0, 0)
            presum1(0, 0, 0, 0, "pool")
            v_mm(0, 0, 0)
            s_half(0, 0, 0, 1)
            presum1(0, 0, 0, 1, "dve")
            v_mm(0, 0, 1)
            s_half(0, 0, 1, 0)
            presum1(0, 0, 1, 0, "pool")
            s_half(0, 0, 1, 1)
            presum1(0, 0, 1, 1, "dve")

            # ---------------- steady loop over the 16 pairs ----------------
            # iteration i handles: S/exp quanta of pair i (i>0; pair 0's were
            # emitted in the prologue), K/V projections for pair i+1 (one
            # iteration of lookahead keeps the PE filler even across ALL
            # iterations, so the exp<->S chain never runs bare), the sp=c
            # block of qtall(g+1), and the softmax tail of pairs i-1/i-2.
            # Slot layout between the four exp quanta (each block <=0.8us so
            # no slot delays the next quantum's matmuls beyond the exp pace):
            #   A: O-par1(i-2) + its normalize     B: K(i+1) + V-half(i+1)
            #   C: qtall block + colsum(i-1)       D: V-half(i+1) + O-par0(i-1)
            pairs = [(g, c) for g in range(4) for c in range(4)]
            for i, (g, c) in enumerate(pairs):
                prev = pairs[i - 1] if i > 0 else None
                pv2 = pairs[i - 2] if i > 1 else None
                nxt = pairs[i + 1] if i < 15 else None
                last = (i == 15)
                j0eng = "dve" if last else "pool"
                if last:
                    # iter 15 has no projection filler left; keep PE warm
                    # between the exp quanta with dummy matmuls into the
                    # now-idle pp psum pool (p-state decay would otherwise
                    # stall the exp<->S chain ~1.7us here, measured)
                    dummy = pp_psum.tile([128, 512], FP, tag="pp",
                                         name="dummy")
                if prev is not None:
                    presum2(*prev)                     # DVE (early)
                    s_half(g, c, 0, 0)                 # PE 2mm + exp
                    presum1(g, c, 0, 0, j0eng)
                if nxt is not None:
                    k_proj(*nxt)                       # PE 4mm + DVE drain
                if last:
                    for _ in range(3):
                        nc.tensor.matmul(dummy[0:64, :], ones[:, 0:A],
                                         warmsrc[:], start=True, stop=True,
                                         skip_group_check=True)
                if prev is not None:
                    s_half(g, c, 0, 1)                 # PE 2mm + exp
                    presum1(g, c, 0, 1, "dve")
                if nxt is not None:
                    v_mm(*nxt, 0)                      # PE 4mm
                    v_mm(*nxt, 1)                      # PE 4mm + drains
                if last:
                    for _ in range(3):
                        nc.tensor.matmul(dummy[0:64, :], ones[:, 0:A],
                                         warmsrc[:], start=True, stop=True,
                                         skip_group_check=True)
                if last:
                    # pre-add for the final (3,3) output early: acc2[1]'s
                    # last update was pair (3,1), normalized last iteration.
                    # preB from the const pool (misc rotation would reuse it).
                    preB = const.tile([128, 512], FP, tag="preB")
                    nc.vector.tensor_add(preB[:], acc2[1][:], prt2[:, 1, :])
                if prev is not None:
                    s_half(g, c, 1, 0)                 # PE 2mm + exp
                    presum1(g, c, 1, 0, j0eng)
                if g < 3:
                    q_proj2(g + 1, c)                  # PE 8mm + DVE drain
                if g == 3 and c < 2:
                    wr_proj(c)                         # PE 4mm + DVE drain
                if prev is not None:
                    colsum(*prev)                      # PE 2mm
                    recip(*prev)                       # DVE
                    s_half(g, c, 1, 1)                 # PE 2mm + exp
                    presum1(g, c, 1, 1, "dve")
                    o_mm_p(*prev, 0)                   # PE 4mm
                    o_mm_p(*prev, 1)                   # PE 4mm
                    normalize(*prev)                   # DVE mul + pool add
                if last:
                    # acc2[0] complete after normalize(3, 2)
                    epilogue(0)

            # ---------------- tail: pair (3,3), latency-optimized ----------
            presum2(3, 3)
            es = es_t.pop((3, 3))
            vsb = vsb_t.pop((3, 3))
            e1 = esum1_t.pop((3, 3))
            sumb2 = so_psum.tile([128, 512], FP, tag="so", name="sumb2")
            for par in range(2):
                nc.tensor.matmul(sumb2[64 * par:64 * par + 64, :],
                                 ones[:, 0:A], e1[par][:],
                                 start=True, stop=True,
                                 skip_group_check=True)
            ot2 = so_psum.tile([128, 512], FP, tag="so", name="ot2")
            for kn in range(4):
                for par in range(2):
                    nc.tensor.matmul(ot2[64 * par:64 * par + 64, :],
                                     vsb[:, kn, par, :], es[par][:, kn, :],
                                     start=(kn == 0), stop=(kn == 3),
                                     skip_group_check=True)
            recipb2 = misc_pool.tile([128, 512], FP, tag="recip",
                                     name="recipB")
            otmp2 = misc_pool.tile([128, 512], FP, tag="otmp", name="otmpB")
            outsb2 = misc_pool.tile([128, 512], F16, tag="outsb",
                                    name="outsbB")
            for lo, hi in ((0, 256), (256, 512)):
                nc.vector.reciprocal_approx_fast(out=recipb2[:, lo:hi],
                                                 in_=sumb2[:, lo:hi])
                nc.vector.tensor_mul(otmp2[:, lo:hi], ot2[:, lo:hi],
                                     recipb2[:, lo:hi])
                nc.vector.tensor_add(otmp2[:, lo:hi], preB[:, lo:hi],
                                     otmp2[:, lo:hi])
                nc.vector.tensor_scalar_max(outsb2[:, lo:hi],
                                            otmp2[:, lo:hi], 0.0)
                for par in range(2):
                    nc.sync.dma_start(
                        out=out_e[:, 512 * (2 + par) + lo:512 * (2 + par) + hi],
                        in_=outsb2[64 * par:64 * par + 64, lo:hi])

    nc.finalize()
    return nc


def _stage_inputs(x, Wq, Wk, Wv, Wr):
    """Build per-core input dicts."""
    Wk_perm = np.ascontiguousarray(Wk[:, _M_OF_P].astype(np.float16))
    Wv8 = np.ascontiguousarray((Wv / 8.0).astype(np.float16))
    Wq_c = np.ascontiguousarray(Wq.astype(np.float16))
    Wr_c = np.ascontiguousarray(Wr.astype(np.float16))
    in_maps = []
    for d in range(NCORES):
        xa = np.concatenate(
            [x[4 * h + d // 2, 256 * (d % 2):256 * (d % 2) + 256, :]
             for h in range(H)], axis=0)
        xaT = np.ascontiguousarray(xa.T.astype(np.float16))
        xoT = np.ascontiguousarray(
            np.concatenate([x[4 * d + i][_M_OF_P, :].T for i in range(4)],
                           axis=1).astype(np.float16))
        in_maps.append({
            "xaT": xaT, "xoT": xoT,
            "wqk": np.concatenate([Wq_c, Wk_perm], axis=1),
            "wvr": np.concatenate([Wv8, Wr_c], axis=1),
        })
    return in_maps


_CACHED = {}


def kernel(x, Wq, Wk, Wv, Wr, _want_trace=False):
    from concourse.bass_utils import run_bass_kernel_spmd

    x = np.asarray(x, dtype=np.float32)
    in_maps = _stage_inputs(x, np.asarray(Wq, np.float32),
                            np.asarray(Wk, np.float32),
                            np.asarray(Wv, np.float32),
                            np.asarray(Wr, np.float32))

    if "nc" not in _CACHED:
        _CACHED["nc"] = build_core_graph()
    nc = _CACHED["nc"]

    res = run_bass_kernel_spmd(nc, in_maps, core_ids=list(range(NCORES)),
                               trace=_want_trace)
    _CACHED["last_result"] = res

    out = np.zeros((B, M, A), np.float32)
    for d in range(NCORES):
        o = res.results[d]["out"].astype(np.float32)  # (64, 2048)
        for i in range(4):
            out[4 * d + i] = o[:, 512 * i + _P_OF_M].T
    return out


if __name__ == "__main__":
    np.random.seed(0)
    pass


# revision 34
# speedup vs baseline: 1.0186x; 1.0186x over previous
"""Trainium2 Bass kernel for nn_Attention (buggy-reshape attention), 8-core SPMD.

Math (reference): q/k/v = (x @ W).reshape entangles batch and head. Each of the
256 (h,b) "chunks" is a contiguous 64-row block of the projected (16384, 512)
matrices:
  K_c = XK[64c:64c+64, :]            (64=A, 512=M)  -- used as-is
  Q_c = XQ[64c:64c+64, :].reshape(512, 64)
  V_c = XV[64c:64c+64, :].reshape(512, 64)
  out_c = softmax(Q_c @ K_c, -1) @ V_c ; final[b] = relu(mean_h out_(h,b) + x_b @ Wr)

Chunk (h,b) touches only x[4h + b//8, 64*(b%8):64*(b%8)+64, :]. We shard by
OUTPUT batch: core d owns batches 4d..4d+3 (all 8 heads) and is handed exactly
the x rows it needs -> zero collectives; head-mean is local.

Per-core layouts (m-permutation p = 64*s + r where m = 8*r + s; same perm used
for the n axis via host-permuted Wk columns):
  S^T tiles (n'-part, p-free) = Ksb_slice.T @ QTall_slice ; softmax over n'
  (partition axis; no max subtraction -- scores are O(+-50), exp fits fp32
  easily); column sums via ones-matmul; O^T = V_perm.T @ expS, normalized by
  approx-reciprocal broadcast; 1/8 head-mean folded into Wv.

Schedule: the Act engine's exp stream is the metronome (4 x [128,1024] exps
per chunk-pair, ~4.2us -- Act has no 2x mode).  Iteration i emits pair i's 8
S-matmuls as four 2-matmul quanta, each immediately followed by its exp and
separated by filler blocks of other PE work, laid out as
  [s1][K(i+1)][s2][V(i+1)][s3][qtall blk / Wr][colsum(i-1)][s4][O(i-1)]
K/V are projected one iteration ahead so EVERY iteration (incl. group 3,
which has no qtall blocks) carries filler -- a bare iteration lets the PE
p-state decay and the exp<->S chain stall.  exp pre-sums: (kn0+kn1) on the
otherwise-idle gpsimd (each depends on a single exp; ~1.25us/add at 0.42
sw-efficiency), (kn2+kn3) on DVE, folded by DVE at the next iteration top so
the column-sum is a single accumulation chain per chunk.  The accumulator
add of the normalize runs on gpsimd (sbuf-only); all psum reads stay on
Act/DVE (gpsimd cannot touch psum, and its max-with-cast is ~7us -- never
use it).  A 12-matmul PE warmup during the input-DMA wait ramps the p-state
to 2.4GHz before the first projection; the prologue Q blocks alternate
between the pp and so psum pools so their drains don't serialize the 2-buf
rotation.  The final pair's softmax tail runs entirely on DVE at 256-column
granularity (the gpsimd adds would queue ahead of it).

dtypes: everything fp16 into the PE except the exp output (bf16, needs range)
and so the V/ones stationaries of the O/sum matmuls are bf16 to match es;
psum always fp32; final output fp16 (cast to fp32 on host).  M=64 matmuls are
column-paired onto disjoint PE halves via psum base-partition-64 outputs (auto
tile_position); two accumulation chains never interleave within the same psum
(bank, partition-half).  Row-pairing the M=512 S-matmuls is deliberately
avoided, and so is packing PE much beyond ~93% busy: both trip the chip power
governor (~20% GLOBAL clock drop, measured -- exp slices go 1114 -> 1335ns).
"""

import os
import sys

import numpy as np

sys.path.insert(0, "/opt/trn_rl_repo")

import concourse.bass as bass
import concourse.bacc as bacc
import concourse.mybir as mybir
from concourse.tile import TileContext

FP = mybir.dt.float32
BF = mybir.dt.bfloat16
F16 = mybir.dt.float16
AF = mybir.ActivationFunctionType
ALU = mybir.AluOpType

B, M, E, H, A = 32, 512, 256, 8, 64
NCORES = 8

# m (and n) permutation: p = 64*s + r  <->  m = 8*r + s
_M_OF_P = np.array([8 * (p % 64) + p // 64 for p in range(512)])
_P_OF_M = np.array([64 * (m % 8) + m // 8 for m in range(512)])


def build_core_graph():
    nc = bacc.Bacc(target_bir_lowering=False)

    xaT_e = nc.declare_dram_parameter("xaT", [E, 2048], F16, isOutput=False)
    xoT_e = nc.declare_dram_parameter("xoT", [E, 2048], F16, isOutput=False)
    wqk_e = nc.declare_dram_parameter("wqk", [E, 1024], F16, isOutput=False)
    wvr_e = nc.declare_dram_parameter("wvr", [E, 512 + A], F16, isOutput=False)
    out_e = nc.declare_dram_parameter("out", [A, 2048], F16, isOutput=True)

    with TileContext(nc) as tc:
        from contextlib import ExitStack

        with ExitStack() as ctx:
            const = ctx.enter_context(tc.tile_pool(name="const", bufs=1))
            qt_pool = ctx.enter_context(tc.tile_pool(name="qt", bufs=2))
            ksb_pool = ctx.enter_context(tc.tile_pool(name="ksb", bufs=8))
            vsb_pool = ctx.enter_context(tc.tile_pool(name="vsb", bufs=8))
            exps_pool = ctx.enter_context(tc.tile_pool(name="exps", bufs=4))
            esum_pool = ctx.enter_context(tc.tile_pool(name="esum", bufs=4))
            esum1_pool = ctx.enter_context(tc.tile_pool(name="esum1", bufs=2))
            misc_pool = ctx.enter_context(tc.tile_pool(name="misc", bufs=4))
            acc_pool = ctx.enter_context(tc.tile_pool(name="acc", bufs=2))

            pp_psum = ctx.enter_context(tc.tile_pool(name="pp", bufs=2, space="PSUM"))
            st_psum = ctx.enter_context(tc.tile_pool(name="st", bufs=2, space="PSUM"))
            so_psum = ctx.enter_context(tc.tile_pool(name="so", bufs=2, space="PSUM"))

            # ---- input tiles ----
            xaT_t = const.tile([128, 2, 4, 4, 2, A], F16, tag="xaT")
            xoT_t = const.tile([128, 2, 2048], F16, tag="xoT")
            wqk = const.tile([128, 2, 1024], F16, tag="wqk")
            wvr = const.tile([128, 2, 512 + A], F16, tag="wvr")
            ones = const.tile([128, A], BF, tag="ones")
            nc.vector.memset(ones[:], 1.0)

            # startup-critical loads, finest first: the first K matmul needs
            # only wkp + xaT(g0,c0); the par0 Q projection then needs wq +
            # the rest of xaT g0.  wvr (V/Wr weights) follows; bulk (xaT
            # g1:3, xoT) is gated behind group-0 compute below so it cannot
            # steal DMA engines from these.
            nc.sync.dma_start(
                out=xaT_t[:, :, 0, 0, :, :],
                in_=xaT_e[:, 0:128].rearrange("(k p) (t r) -> p k t r",
                                              k=2, t=2))
            nc.scalar.dma_start(
                out=wqk[:, :, 512:1024],
                in_=wqk_e[:, 512:1024].rearrange("(k p) n -> p k n", k=2))
            nc.sync.dma_start(
                out=wqk[:, :, 0:512],
                in_=wqk_e[:, 0:512].rearrange("(k p) n -> p k n", k=2))
            nc.sync.dma_start(
                out=xaT_t[:, :, 0, 1:4, :, :],
                in_=xaT_e[:, 128:512].rearrange("(k p) (c t r) -> p k c t r",
                                                k=2, c=3, t=2))
            nc.scalar.dma_start(
                out=wvr[:, :, :],
                in_=wvr_e[:, :].rearrange("(k p) n -> p k n", k=2))

            wq = [wqk[:, k, 0:512] for k in range(2)]
            wkp = [wqk[:, k, 512:1024] for k in range(2)]
            wv8 = [wvr[:, k, 0:512] for k in range(2)]
            wv8v = [wv8[k].rearrange("p (hh sp a) -> p hh sp a", hh=4, sp=2)
                    for k in range(2)]
            wr = [wvr[:, k, 512:512 + A] for k in range(2)]
            xoT = [xoT_t[:, k, :] for k in range(2)]

            acc2 = [acc_pool.tile([128, 512], FP, tag="acc", name="acc")
                    for _ in range(2)]
            prt2 = const.tile([128, 2, 512], FP, tag="prt2")

            qtall_t = {}
            ksb_t = {}
            vsb_t = {}
            es_t = {}
            esum_t = {}
            esum1_t = {}
            sum_ps = {}

            # ---------------- building blocks ----------------
            def q_proj_par(g, sp, par, drain):
                # one psum bank holds two s-blocks (sh) of one par half; the
                # two accumulation chains are strictly sequential and one
                # copy drains both -> half the psum handoffs.
                if sp == 0 and par == 0:
                    qtall_t[g] = qt_pool.tile([128, 8, 4, A], F16, tag="qt",
                                              name="qtall")
                qp4 = pp_psum.tile([128, 2, 4, A], FP, tag="pp", name="qp4")
                for sh in range(2):
                    s = 2 * sp + sh
                    for k in range(2):
                        nc.tensor.matmul(
                            qp4[64 * par:64 * par + 64, sh, :, :],
                            wq[k][:, 64 * s:64 * s + 64],
                            xaT_t[:, k, g, :, par, :],
                            start=(k == 0), stop=(k == 1),
                            skip_group_check=True)
                dst = qtall_t[g][64 * par:64 * par + 64, 2 * sp:2 * sp + 2]
                src = qp4[64 * par:64 * par + 64, :, :, :]
                if drain == "act":
                    nc.scalar.copy(dst, src)
                else:
                    nc.vector.tensor_copy(dst, src)

            def q_proj2(g, sp, drain="dve", pool=None):
                # both par halves in one psum tile, column-paired matmuls
                if sp == 0:
                    qtall_t[g] = qt_pool.tile([128, 8, 4, A], F16, tag="qt",
                                              name="qtall")
                pool = pool or pp_psum
                qp4 = pool.tile([128, 2, 4, A], FP,
                                tag="pp" if pool is pp_psum else "so",
                                name="qp4")
                for sh in range(2):
                    s = 2 * sp + sh
                    for k in range(2):
                        for par in range(2):
                            nc.tensor.matmul(
                                qp4[64 * par:64 * par + 64, sh, :, :],
                                wq[k][:, 64 * s:64 * s + 64],
                                xaT_t[:, k, g, :, par, :],
                                start=(k == 0), stop=(k == 1),
                                skip_group_check=True)
                if drain == "act":
                    nc.scalar.copy(qtall_t[g][:, 2 * sp:2 * sp + 2, :, :],
                                   qp4[:, :, :, :])
                else:
                    nc.vector.tensor_copy(
                        qtall_t[g][:, 2 * sp:2 * sp + 2, :, :],
                        qp4[:, :, :, :])

            def k_proj(g, c):
                kp2 = pp_psum.tile([128, 512], FP, tag="pp", name="kp2")
                for k in range(2):
                    for par in range(2):
                        nc.tensor.matmul(kp2[64 * par:64 * par + 64, :],
                                         xaT_t[:, k, g, c, par, :], wkp[k],
                                         start=(k == 0), stop=(k == 1),
                                         skip_group_check=True)
                ksb_t[(g, c)] = ksb_pool.tile([128, 512], F16, tag="ksb",
                                              name="ksb")
                nc.vector.tensor_copy(ksb_t[(g, c)][:], kp2[:])

            pv_ps = {}

            def v_mm(g, c, phase):
                # psum lands directly in V_perm layout (sp -> partition half,
                # par -> free dim); chains (0,0)/(1,1) complete before
                # (0,1)/(1,0) so no psum partition-half hosts two chains.
                # Split in two 4mm phases so the block packs into slim slots.
                if phase == 0:
                    pv_ps[(g, c)] = pp_psum.tile([128, 4, 2, A], FP,
                                                 tag="pp", name="pv4")
                pv4 = pv_ps[(g, c)]
                grp = (((0, 0), (1, 1)), ((0, 1), (1, 0)))[phase]
                for k in range(2):
                    for par, sp in grp:
                        nc.tensor.matmul(
                            pv4[64 * sp:64 * sp + 64, :, par, :],
                            xaT_t[:, k, g, c, par, :],
                            wv8v[k][:, :, sp, :],
                            start=(k == 0), stop=(k == 1),
                            skip_group_check=True)
                if phase == 1:
                    pv_ps.pop((g, c))
                    vsb = vsb_pool.tile([128, 4, 2, A], BF, tag="vsb",
                                        name="vsb")
                    nc.scalar.copy(vsb[:, 0:2], pv4[:, 0:2])
                    nc.vector.tensor_copy(vsb[:, 2:4], pv4[:, 2:4])
                    vsb_t[(g, c)] = vsb

            def s_half(g, c, par, half):
                # 2 S^T matmuls + 1 exp for one (par, half).  Unpaired on the
                # PE row halves on purpose: pairing trips the chip power
                # governor (~20% global clock drop, measured).
                if par == 0 and half == 0:
                    es_t[(g, c)] = {}
                if half == 0:
                    es = exps_pool.tile([128, 4, 512], BF, tag="exps",
                                        name="es")
                    es_t[(g, c)][par] = es
                else:
                    es = es_t[(g, c)][par]
                ksb = ksb_t[(g, c)]
                qtall = qtall_t[g]
                st = st_psum.tile([128, 2, 512], FP, tag="st", name="st")
                for q2 in range(2):
                    kn = 2 * half + q2
                    nc.tensor.matmul(
                        st[:, q2, :],
                        ksb[64 * par:64 * par + 64,
                            128 * kn:128 * kn + 128],
                        qtall[64 * par:64 * par + 64, :, c, :],
                        start=True, stop=True)
                nc.scalar.activation(es[:, 2 * half:2 * half + 2, :],
                                     st[:], AF.Exp)

            def presum1(g, c, par, half, engine):
                # esum[:, half, :] = es[:, 2h, :] + es[:, 2h+1, :] -- depends
                # on exactly one exp; runs on gpsimd (idle) in steady state.
                if half == 0:
                    esum_t.setdefault((g, c), {})[par] = esum_pool.tile(
                        [128, 2, 512], BF, tag="esum", name="esum")
                es = es_t[(g, c)][par]
                esum = esum_t[(g, c)][par]
                eng = nc.gpsimd if engine == "pool" else nc.vector
                eng.tensor_add(esum[:, half, :], es[:, 2 * half, :],
                               es[:, 2 * half + 1, :])

            def presum2(g, c):
                # fold to a single kn chunk on DVE -> single-chain column sum
                esum1_t[(g, c)] = {}
                for par in range(2):
                    esum = esum_t[(g, c)].pop(par)
                    e1 = esum1_pool.tile([128, 512], BF, tag="esum1",
                                         name="esum1")
                    esum1_t[(g, c)][par] = e1
                    nc.vector.tensor_add(e1[:], esum[:, 0, :], esum[:, 1, :])

            def colsum(g, c):
                sumb2 = so_psum.tile([128, 512], FP, tag="so", name="sumb2")
                sum_ps[(g, c)] = sumb2
                e1 = esum1_t.pop((g, c))
                for par in range(2):
                    nc.tensor.matmul(sumb2[64 * par:64 * par + 64, :],
                                     ones[:, 0:A], e1[par][:],
                                     start=True, stop=True,
                                     skip_group_check=True)

            rec_t = {}

            def recip(g, c):
                sumb2 = sum_ps[(g, c)]
                recipb2 = misc_pool.tile([128, 512], FP, tag="recip",
                                         name="recipb2")
                nc.vector.reciprocal_approx_fast(out=recipb2[:],
                                                 in_=sumb2[:])
                rec_t[(g, c)] = recipb2

            ot_ps = {}

            def o_mm_p(g, c, par):
                # one par's 4mm accumulation chain; par 0 runs at the end of
                # one iteration, par 1 at the start of the next, so the O
                # block never fattens a single inter-quantum slot.
                if par == 0:
                    ot_ps[(g, c)] = so_psum.tile([128, 512], FP, tag="so",
                                                 name="ot2")
                ot2 = ot_ps[(g, c)]
                es = es_t[(g, c)][par]
                vsb = vsb_t[(g, c)]
                for kn in range(4):
                    nc.tensor.matmul(ot2[64 * par:64 * par + 64, :],
                                     vsb[:, kn, par, :], es[:, kn, :],
                                     start=(kn == 0), stop=(kn == 3),
                                     skip_group_check=True)
                if par == 1:
                    es_t.pop((g, c))
                    vsb_t.pop((g, c))

            def normalize(g, c):
                ot2 = ot_ps.pop((g, c))
                recipb2 = rec_t.pop((g, c))
                q = c % 2
                first = (g == 0 and c < 2)
                sum_ps.pop((g, c))
                if first:
                    nc.vector.tensor_mul(acc2[q][:], ot2[:], recipb2[:])
                else:
                    # mul reads psum (DVE only); the accumulate is sbuf-only
                    # and runs on gpsimd, which has slack
                    otmp2 = misc_pool.tile([128, 512], FP, tag="otmp",
                                           name="otmp2")
                    nc.vector.tensor_mul(otmp2[:], ot2[:], recipb2[:])
                    eng = nc.vector if (g, c) == (3, 2) else nc.gpsimd
                    eng.tensor_add(acc2[q][:], acc2[q][:], otmp2[:])

            def wr_proj(q):
                rp2 = pp_psum.tile([128, 512], FP, tag="pp", name="rp2")
                for k in range(2):
                    for par in range(2):
                        nc.tensor.matmul(
                            rp2[64 * par:64 * par + 64, :],
                            wr[k],
                            xoT[k][:, 512 * (2 * q + par):
                                   512 * (2 * q + par) + 512],
                            start=(k == 0), stop=(k == 1),
                            skip_group_check=True)
                nc.vector.tensor_copy(prt2[:, q, :], rp2[:])

            def epilogue(q):
                # acc2[q] is complete; pre-add on gpsimd (sbuf-only), relu on
                # DVE (gpsimd max with dtype cast is pathologically slow)
                outsb2 = misc_pool.tile([128, 512], F16, tag="outsb",
                                        name="outsb2")
                pre2 = misc_pool.tile([128, 512], FP, tag="pre", name="pre2")
                nc.gpsimd.tensor_add(pre2[:], acc2[q][:], prt2[:, q, :])
                nc.vector.tensor_scalar_max(outsb2[:], pre2[:], 0.0)
                for par in range(2):
                    nc.gpsimd.dma_start(
                        out=out_e[:, 512 * (2 * q + par):
                                  512 * (2 * q + par) + 512],
                        in_=outsb2[64 * par:64 * par + 64, :])

            # ---------------- prologue: pair (0,0) + group-0 projections ----
            # PE p-state warmup during the input-DMA wait: the ramp reaches
            # 2.4GHz only after ~3us of continuous execution, so burn dummy
            # matmuls (never read) while the weights stream in; the real
            # projections then start at full clock.
            warmsrc = const.tile([128, 512], BF, tag="warmsrc")
            nc.vector.memset(warmsrc[:], 1.0)
            warm = so_psum.tile([128, 512], FP, tag="so", name="warm")
            for _ in range(12):
                nc.tensor.matmul(warm[0:64, :], ones[:, 0:A], warmsrc[:],
                                 start=True, stop=True, skip_group_check=True)
            k_proj(0, 0)
            for sp in range(4):
                # alternate the psum pool so the four blocks aren't gated on
                # each other's drains through the 2-buf pp rotation
                q_proj2(0, sp, drain="act" if sp % 2 == 0 else "dve",
                        pool=pp_psum if sp % 2 == 0 else so_psum)
                if sp == 0:
                    # Bulk loads gated behind early group-0 compute via junk
                    # WAW stores (overwritten by the DMAs) so their transfers
                    # don't steal DMA engines from the startup-critical ones.
                    nc.gpsimd.tensor_copy(xaT_t[0:1, 0, 1, 0, 0, 0:4],
                                          qtall_t[0][0:1, 0, 0, 0:4])
                    nc.gpsimd.tensor_copy(xoT_t[0:1, 0, 0:4],
                                          qtall_t[0][0:1, 0, 0, 0:4])
                    nc.gpsimd.dma_start(
                        out=xaT_t[:, :, 1:4, :, :, :],
                        in_=xaT_e[:, 512:2048].rearrange(
                            "(k p) (g c t r) -> p k g c t r", k=2, g=3, c=4,
                            t=2))
                    nc.gpsimd.dma_start(
                        out=xoT_t[:, :, :],
                        in_=xoT_e[:, :].rearrange("(k p) n -> p k n", k=2))
            # first exps fire as early as possible; pair (0,0)'s V
            # projection fills PE between the first quanta
            s_half(0, 0, 0, 0)
            presum1(0, 0, 0, 0, "pool")
            v_mm(0, 0, 0)
            s_half(0, 0, 0, 1)
            presum1(0, 0, 0, 1, "dve")
            v_mm(0, 0, 1)
            s_half(0, 0, 1, 0)
            presum1(0, 0, 1, 0, "pool")
            s_half(0, 0, 1, 1)
            presum1(0, 0, 1, 1, "dve")

            # ---------------- steady loop over the 16 pairs ----------------
            # iteration i handles: S/exp quanta of pair i (i>0; pair 0's were
            # emitted in the prologue), K/V projections for pair i+1 (one
            # iteration of lookahead keeps the PE filler even across ALL
            # iterations, so the exp<->S chain never runs bare), the sp=c
            # block of qtall(g+1), and the softmax tail of pairs i-1/i-2.
            # Slot layout between the four exp quanta (each block <=0.8us so
            # no slot delays the next quantum's matmuls beyond the exp pace):
            #   A: O-par1(i-2) + its normalize     B: K(i+1) + V-half(i+1)
            #   C: qtall block + colsum(i-1)       D: V-half(i+1) + O-par0(i-1)
            pairs = [(g, c) for g in range(4) for c in range(4)]
            for i, (g, c) in enumerate(pairs):
                prev = pairs[i - 1] if i > 0 else None
                pv2 = pairs[i - 2] if i > 1 else None
                nxt = pairs[i + 1] if i < 15 else None
                last = (i == 15)
                j0eng = "dve" if last else "pool"
                if prev is not None:
                    presum2(*prev)                     # DVE (early)
                    s_half(g, c, 0, 0)                 # PE 2mm + exp
                    presum1(g, c, 0, 0, j0eng)
                if nxt is not None:
                    k_proj(*nxt)                       # PE 4mm + DVE drain
                if prev is not None:
                    s_half(g, c, 0, 1)                 # PE 2mm + exp
                    presum1(g, c, 0, 1, "dve")
                if nxt is not None:
                    v_mm(*nxt, 0)                      # PE 4mm
                    v_mm(*nxt, 1)                      # PE 4mm + drains
                if last:
                    # pre-add for the final (3,3) output early: acc2[1]'s
                    # last update was pair (3,1), normalized last iteration.
                    # preB from the const pool (misc rotation would reuse it).
                    preB = const.tile([128, 512], FP, tag="preB")
                    nc.vector.tensor_add(preB[:], acc2[1][:], prt2[:, 1, :])
                if prev is not None:
                    s_half(g, c, 1, 0)                 # PE 2mm + exp
                    presum1(g, c, 1, 0, j0eng)
                if g < 3:
                    q_proj2(g + 1, c)                  # PE 8mm + DVE drain
                if g == 3 and c < 2:
                    wr_proj(c)                         # PE 4mm + DVE drain
                if prev is not None:
                    colsum(*prev)                      # PE 2mm
                    recip(*prev)                       # DVE
                    s_half(g, c, 1, 1)                 # PE 2mm + exp
                    presum1(g, c, 1, 1, "dve")
                    o_mm_p(*prev, 0)                   # PE 4mm
                    o_mm_p(*prev, 1)                   # PE 4mm
                    normalize(*prev)                   # DVE mul + pool add
                if last:
                    # acc2[0] complete after normalize(3, 2)
                    epilogue(0)

            # ---------------- tail: pair (3,3), latency-optimized ----------
            presum2(3, 3)
            es = es_t.pop((3, 3))
            vsb = vsb_t.pop((3, 3))
            e1 = esum1_t.pop((3, 3))
            sumb2 = so_psum.tile([128, 512], FP, tag="so", name="sumb2")
            for par in range(2):
                nc.tensor.matmul(sumb2[64 * par:64 * par + 64, :],
                                 ones[:, 0:A], e1[par][:],
                                 start=True, stop=True,
                                 skip_group_check=True)
            ot2 = so_psum.tile([128, 512], FP, tag="so", name="ot2")
            for kn in range(4):
                for par in range(2):
                    nc.tensor.matmul(ot2[64 * par:64 * par + 64, :],
                                     vsb[:, kn, par, :], es[par][:, kn, :],
                                     start=(kn == 0), stop=(kn == 3),
                                     skip_group_check=True)
            recipb2 = misc_pool.tile([128, 512], FP, tag="recip",
                                     name="recipB")
            otmp2 = misc_pool.tile([128, 512], FP, tag="otmp", name="otmpB")
            outsb2 = misc_pool.tile([128, 512], F16, tag="outsb",
                                    name="outsbB")
            for lo, hi in ((0, 256), (256, 512)):
                nc.vector.reciprocal_approx_fast(out=recipb2[:, lo:hi],
                                                 in_=sumb2[:, lo:hi])
                nc.vector.tensor_mul(otmp2[:, lo:hi], ot2[:, lo:hi],
                                     recipb2[:, lo:hi])
                nc.vector.tensor_add(otmp2[:, lo:hi], preB[:, lo:hi],
                                     otmp2[:, lo:hi])
                nc.vector.tensor_scalar_max(outsb2[:, lo:hi],
                                            otmp2[:, lo:hi], 0.0)
                for par in range(2):
                    nc.sync.dma_start(
                        out=out_e[:, 512 * (2 + par) + lo:512 * (2 + par) + hi],
                        in_=outsb2[64 * par:64 * par + 64, lo:hi])

    nc.finalize()
    return nc


def _stage_inputs(x, Wq, Wk, Wv, Wr):
    """Build per-core input dicts."""
    Wk_perm = np.ascontiguousarray(Wk[:, _M_OF_P].astype(np.float16))
    Wv8 = np.ascontiguousarray((Wv / 8.0).astype(np.float16))
    Wq_c = np.ascontiguousarray(Wq.astype(np.float16))
    Wr_c = np.ascontiguousarray(Wr.astype(np.float16))
    in_maps = []
    for d in range(NCORES):
        xa = np.concatenate(
            [x[4 * h + d // 2, 256 * (d % 2):256 * (d % 2) + 256, :]
             for h in range(H)], axis=0)
        xaT = np.ascontiguousarray(xa.T.astype(np.float16))
        xoT = np.ascontiguousarray(
            np.concatenate([x[4 * d + i][_M_OF_P, :].T for i in range(4)],
                           axis=1).astype(np.float16))
        in_maps.append({
            "xaT": xaT, "xoT": xoT,
            "wqk": np.concatenate([Wq_c, Wk_perm], axis=1),
            "wvr": np.concatenate([Wv8, Wr_c], axis=1),
        })
    return in_maps


_CACHED = {}


def kernel(x, Wq, Wk, Wv, Wr, _want_trace=False):
    from concourse.bass_utils import run_bass_kernel_spmd

    x = np.asarray(x, dtype=np.float32)
    in_maps = _stage_inputs(x, np.asarray(Wq, np.float32),
                            np.asarray(Wk, np.float32),
                            np.asarray(Wv, np.float32),
                            np.asarray(Wr, np.float32))

    if "nc" not in _CACHED:
        _CACHED["nc"] = build_core_graph()
    nc = _CACHED["nc"]

    res = run_bass_kernel_spmd(nc, in_maps, core_ids=list(range(NCORES)),
                               trace=_want_trace)
    _CACHED["last_result"] = res

    out = np.zeros((B, M, A), np.float32)
    for d in range(NCORES):
        o = res.results[d]["out"].astype(np.float32)  # (64, 2048)
        for i in range(4):
            out[4 * d + i] = o[:, 512 * i + _P_OF_M].T
    return out


if __name__ == "__main__":
    np.random.seed(0)
    pass
